# revision 42
# baseline (speedup 1.0000x reference)
"""Bi-directional minGRU kernel for Trainium2 (8 NeuronCores, Bass/Tile).

Strategy
--------
Data-parallel over batch: B=256 examples sharded 32 per core. Per example all
tensors live in feature-major layout [feature->partition, time->free], so every
linear layer is a TensorE matmul with K=features on partitions, and the minGRU
recurrence is a single hardware `tensor_tensor_scan` along the free (time)
axis: rows 0-63 carry the forward direction in normal time order, rows 64-127
carry the backward direction in *reversed* time order (the reversal is free:
backward-direction matmul rhs operands are negative-stride views).

Matmuls run in float32r (fp32 bits, 1 PE cycle/row instead of 4) or bf16;
f32r requires tile_position (0,0), so the fwd/bwd gate lhsTs are zero-padded
to M=128 and accumulate into one psum tile.

Host-side (numpy, fp64 then cast to fp32) the linear chains are fused:
    gz_d = (wz@proj[:, :3]) @ x3 + (wz@proj[:, 3:]@te_w2) @ relu_te1 + bias
so the time encoder's second layer, the input projections, and the gate
weights collapse into single [67 -> 64] matmuls, and the head's te branch
collapses into W1t2 = gh_w1[:,128:] @ te_w2.

The masked-position fixup  h_apply = m*pre + (1-m)*final  commutes with the
head matmul (m is a per-time scalar):  W@h_apply = W@(m*(pre-final)) + W@final.
The kernel builds Dn = (pre - final) with the BACKWARD half re-reversed into
normal time order (free negative-stride reads), multiplies by the
normal-order mask once (Eh = m*Dn, bf16), and the whole head layer-1 h-branch
is then a single K=128 bf16 matmul; W@final folds into the head bias.

Examples are processed in pairs: the input rows and the mask broadcast for
two examples share one DMA each to amortize the ~2us fixed DMA cost.
"""
import os
import sys

for _p in ("/opt/trn_rl_repo", "/root/.axon_site/_ro/trn_rl_repo"):
    if os.path.isdir(_p) and _p not in sys.path:
        sys.path.insert(0, _p)

import numpy as np
from contextlib import ExitStack

import concourse.bacc as bacc
import concourse.tile as tile
import concourse.mybir as mybir
from concourse.bass_utils import run_bass_kernel_spmd

F32 = mybir.dt.float32
F32R = mybir.dt.float32r
BF16 = mybir.dt.bfloat16
AF = mybir.ActivationFunctionType
OP = mybir.AluOpType


def _f(ap):
    """View an f32r AP as plain fp32 (activation bias operands / fp32 mm)."""
    return ap.bitcast(F32)

B, L, H, TE = 256, 2048, 64, 64
NCORES = 8
BS = B // NCORES          # examples per core
NW = 965                  # packed weight columns (see _pack_weights)

# weight column layout inside the packed [128, NW] f32r tile.  f32r matmuls
# must sit at tile_position (0,0), so each gate lhsT is zero-padded to M=128
# and fwd/bwd accumulate into one psum tile.
_C_W1FB = 0               # lhsT of [W1f | W1b], [128, 128] (fp32 psv matmul)
_C_W1T2 = 128             # lhsT of W1t2, rows 0-63, [64, 128]
_C_ZF = 256               # fwd z lhsT [67, 128], cols 0-63 filled
_C_ZB = 384               # bwd z lhsT [67, 128], cols 64-127 filled
_C_HF = 512               # fwd h lhsT [67, 128], cols 0-63 filled
_C_HB = 640               # bwd h lhsT [67, 128], cols 64-127 filled
_C_A1 = 768               # te_w1 lhsT, rows 64-67 (row 67 = t), [4, 64]
_C_ZBIAS = 832            # gate z bias column [128, 1]
_C_HBIAS = 833            # gate h bias column
_C_HEADB = 834            # head bias column (gh_b1 + W1t@te_b2)
_C_B1 = 835               # te bias, rows 0-63

_cache = {}


def _pack_weights(inp):
    """Fuse the linear chains (fp64) and pack lhsTs into one [128, NW] f32r
    array plus a small bf16 block [128, 129] (W1FB^T | gh_w2)."""
    g = {k: np.asarray(v, np.float64) for k, v in inp.items()}
    wts = np.zeros((128, NW), np.float64)

    def fuse(proj_w, proj_b, wz, bz, wh, bh):
        P3 = proj_w[:, :3]
        Pte_te2 = proj_w[:, 3:] @ g["te_w2"]
        pbias = proj_w[:, 3:] @ g["te_b2"] + proj_b
        # x3 rows on the device are ordered [mask, x1, x2]
        reord = np.stack([P3[:, 2], P3[:, 0], P3[:, 1]], axis=1)
        return (
            np.concatenate([wz @ Pte_te2, wz @ reord], axis=1),  # (64, 67)
            wz @ pbias + bz,
            np.concatenate([wh @ Pte_te2, wh @ reord], axis=1),
            wh @ pbias + bh,
        )

    Zf, zbf, Hf, hbf = fuse(g["fproj_w"], g["fproj_b"], g["fwz"], g["fbz"],
                            g["fwh"], g["fbh"])
    Zb, zbb, Hb, hbb = fuse(g["bproj_w"], g["bproj_b"], g["bwz"], g["bbz"],
                            g["bwh"], g["bbh"])
    # gate lhsT: [K=67 rows: 0-63 r, 64 mask, 65 x1, 66 x2][M=128 zero-padded]
    wts[0:67, _C_ZF:_C_ZF + 64] = Zf.T
    wts[0:67, _C_HF:_C_HF + 64] = Hf.T
    wts[0:67, _C_ZB + 64:_C_ZB + 128] = Zb.T
    wts[0:67, _C_HB + 64:_C_HB + 128] = Hb.T
    wts[0:64, _C_ZBIAS] = zbf
    wts[64:128, _C_ZBIAS] = zbb
    wts[0:64, _C_HBIAS] = hbf
    wts[64:128, _C_HBIAS] = hbb
    # head
    W1f = g["gh_w1"][:, :64]
    W1b = g["gh_w1"][:, 64:128]
    W1t = g["gh_w1"][:, 128:192]
    W1fb = np.concatenate([W1f, W1b], axis=1)          # (128, 128)
    wts[0:128, _C_W1FB:_C_W1FB + 128] = W1fb.T
    wts[0:64, _C_W1T2:_C_W1T2 + 128] = (W1t @ g["te_w2"]).T
    wts[0:128, _C_HEADB] = g["gh_b1"] + W1t @ g["te_b2"]
    # te first layer: lhsT rows 64-67, row 67 = t (rows 64-66 stay zero so
    # the matmul can run K=4 from 32-aligned base partition 64)
    wts[67, _C_A1:_C_A1 + 64] = g["te_w1"][:, 0]
    wts[0:64, _C_B1] = g["te_b1"]
    # bf16 block: head1 h-branch lhsT + head2 weight
    bnp = mybir.dt.np(BF16)
    wtsb = np.zeros((128, 129), bnp)
    wtsb[:, 0:128] = W1fb.T.astype(np.float32).astype(bnp)
    wtsb[:, 128] = g["gh_w2"][0].astype(np.float32).astype(bnp)
    return (np.ascontiguousarray(wts, np.float32), wtsb,
            np.float32(g["gh_b2"][0]))


def _build_program():
    """Build + compile the 8-core SPMD Bass program once."""
    nc = bacc.Bacc("TRN2", num_devices=NCORES, debug=False)
    wts_d = nc.dram_tensor("wts", [128, NW], F32R, kind="ExternalInput")
    wtsb_d = nc.dram_tensor("wtsb", [128, 129], BF16, kind="ExternalInput")
    inx_d = nc.dram_tensor("inx", [BS, 4, L], F32R, kind="ExternalInput")
    mb_d = nc.dram_tensor("mb", [BS, L], BF16, kind="ExternalInput")
    out_d = nc.dram_tensor("out", [BS, L], F32, kind="ExternalOutput")

    with tile.TileContext(nc) as tc, ExitStack() as ctx:
        wpool = ctx.enter_context(tc.tile_pool(name="w", bufs=1))
        ppool = ctx.enter_context(tc.tile_pool(name="pp", bufs=3))
        pool = ctx.enter_context(tc.tile_pool(name="p", bufs=3))
        spool = ctx.enter_context(tc.tile_pool(name="s", bufs=3))
        # role-separated psum pools: a single rotating pool couples the next
        # example's first matmul to this example's last activation (pipeline
        # killer).  te/gates/head1 rotate in 1024-wide tiles (2 banks each,
        # activations cover 1024 columns at once); psv+head2 in 512-wide.
        ps_gate = ctx.enter_context(tc.tile_pool(name="pg", bufs=2, space="PSUM"))
        ps_h1 = ctx.enter_context(tc.tile_pool(name="ph1", bufs=1, space="PSUM"))
        ps_hq = ctx.enter_context(tc.tile_pool(name="phq", bufs=2, space="PSUM"))

        wts = wpool.tile([128, NW], F32R, tag="wts")
        nc.sync.dma_start(wts[:], wts_d.ap()[:])
        wtsb = wpool.tile([128, 129], BF16, tag="wtsb")
        nc.sync.dma_start(wtsb[:], wtsb_d.ap()[:])
        inx = inx_d.ap()
        mb = mb_d.ap()

        for p in range(BS // 2):
            e0 = 2 * p
            # ---- paired input staging --------------------------------
            # xrp rows: 0-63 r, 64 mask, 65 x1, 66 x2, 67 t; two examples
            # side by side in the free axis
            xrp = ppool.tile([128, 2 * L], F32R, tag="xrp")
            nc.sync.dma_start(
                xrp[64:68, :].rearrange("p (j t) -> p j t", j=2),
                inx[e0:e0 + 2, 0:4, :].transpose([1, 0, 2]))
            mp = ppool.tile([128, 2 * L], BF16, tag="mp")
            nc.sync.dma_start(
                mp[:].rearrange("p (j t) -> p j t", j=2),
                mb[e0:e0 + 2, :].unsqueeze(0).broadcast_to((128, 2, L)))
            outS = spool.tile([128, 2 * 512], F32, tag="outS")

            for j in range(2):
                e = e0 + j
                off = j * L
                eg = e % 4
                xre = xrp[0:67, off:off + L]     # this example's gate rhs
                xrev = xre[:, ::-1]

                # ---- time encoder r = relu(A1 @ t + b1) --------------
                # relu on DVE (max-trick) to keep ACT for the sigmoids
                for q in range(2):
                    oq = slice(off + q * 1024, off + (q + 1) * 1024)
                    pst = ps_gate.tile([128, 1024], F32, tag="pg")
                    for h in range(2):
                        hs = slice(h * 512, (h + 1) * 512)
                        ohs = slice(off + q * 1024 + h * 512,
                                    off + q * 1024 + (h + 1) * 512)
                        nc.tensor.matmul(pst[0:64, hs],
                                         wts[64:68, _C_A1:_C_A1 + 64],
                                         xrp[64:68, ohs], start=True,
                                         stop=True, tile_position=(64, 0))
                    nc.vector.tensor_scalar(
                        xrp[0:64, oq], pst[0:64, :],
                        _f(wts[0:64, _C_B1:_C_B1 + 1]), 0.0,
                        OP.add, OP.max)

                # ---- gates -------------------------------------------
                Z = pool.tile([128, L], BF16, tag="Z")
                TH = pool.tile([128, L], BF16, tag="TH")
                for (dst, cf, cb, bias_c, fn) in (
                        (Z, _C_ZF, _C_ZB, _C_ZBIAS, AF.Sigmoid),
                        (TH, _C_HF, _C_HB, _C_HBIAS, AF.Tanh)):
                    for q in range(2):
                        qs = slice(q * 1024, (q + 1) * 1024)
                        psg = ps_gate.tile([128, 1024], F32, tag="pg")
                        for h in range(2):
                            hs = slice(h * 512, (h + 1) * 512)
                            cs = slice(q * 1024 + h * 512,
                                       q * 1024 + (h + 1) * 512)
                            nc.tensor.matmul(psg[:, hs],
                                             wts[0:67, cf:cf + 128],
                                             xre[:, cs], start=True,
                                             stop=False, tile_position=(0, 0))
                            nc.tensor.matmul(psg[:, hs],
                                             wts[0:67, cb:cb + 128],
                                             xrev[:, cs], start=False,
                                             stop=True, tile_position=(0, 0))
                        nc.scalar.activation(dst[:, qs], psg[:], fn,
                                             bias=_f(wts[:, bias_c:bias_c + 1]))

                # ---- scan inputs: a = 1-z (GPSIMD), b = z*th (GPSIMD)
                A = pool.tile([128, L], BF16, tag="A")
                Bt = pool.tile([128, L], BF16, tag="Bt")
                nc.gpsimd.tensor_scalar(A[:], Z[:], -1.0, 1.0, OP.mult, OP.add)
                nc.gpsimd.tensor_tensor(Bt[:], Z[:], TH[:], OP.mult)

                # ---- the scan ----------------------------------------
                Hs = pool.tile([128, L + 1], BF16, tag="Hs")
                nc.vector.memset(Hs[:, 0:1], 0.0)
                nc.vector.tensor_tensor_scan(Hs[:, 1:L + 1], A[:], Bt[:], 0.0,
                                             OP.mult, OP.add)

                # ---- head bias: W1fb @ final + headb (bf16, N=1) -----
                hfin = Hs[:, L - 1:L]
                psv = ps_hq.tile([128, 512], F32, tag="phq")
                nc.tensor.matmul(psv[:, 0:1], wtsb[:, 0:128],
                                 hfin, start=True, stop=True,
                                 tile_position=(0, 0))
                sbb = spool.tile([128, 4], F32, tag="sbb")
                nc.scalar.activation(sbb[:, 0:1], psv[:, 0:1], AF.Identity,
                                     bias=_f(wts[:, _C_HEADB:_C_HEADB + 1]))
                # fp32 staging of the final state (tensor_scalar scalars
                # must be fp32)
                hf32 = spool.tile([128, 1], F32, tag="hf32")
                nc.scalar.activation(hf32[:], hfin, AF.Copy)

                # ---- Dn = pre - final, bwd half re-reversed to normal
                # time order; Eh = mask * Dn (bf16) ---------------------
                Dn = pool.tile([128, L], BF16, tag="Dn")
                nc.vector.tensor_scalar(Dn[0:64, :], Hs[0:64, 0:L],
                                        hf32[0:64, :], None, OP.subtract)
                nc.vector.tensor_scalar(Dn[64:128, :],
                                        Hs[64:128, 0:L][:, ::-1],
                                        hf32[64:128, :], None, OP.subtract)
                Eh = pool.tile([128, L], BF16, tag="Eh")
                # on DVE (all-bf16 SBUF operands hit the 4x mode); keeping it
                # on Pool head-of-line blocks the next example's A/Bt behind
                # this late-chain op
                nc.vector.tensor_tensor(Eh[:], Dn[:], mp[:, off:off + L],
                                        OP.mult)

                # ---- head layer 1 ------------------------------------
                hid = pool.tile([128, L], BF16, tag="hid")
                for q in range(2):
                    qs = slice(q * 1024, (q + 1) * 1024)
                    psS = ps_h1.tile([128, 1024], F32, tag="ph1")
                    for h in range(2):
                        hs = slice(h * 512, (h + 1) * 512)
                        cs = slice(q * 1024 + h * 512,
                                   q * 1024 + (h + 1) * 512)
                        ocs = slice(off + q * 1024 + h * 512,
                                    off + q * 1024 + (h + 1) * 512)
                        nc.tensor.matmul(psS[:, hs],
                                         wtsb[:, 0:128],
                                         Eh[:, cs], start=True, stop=False,
                                         tile_position=(0, 0))
                        nc.tensor.matmul(psS[:, hs],
                                         wts[0:64, _C_W1T2:_C_W1T2 + 128],
                                         xrp[0:64, ocs], start=False,
                                         stop=True, tile_position=(0, 0))
                    nc.scalar.activation(hid[:, qs], psS[:], AF.Relu,
                                         bias=sbb[:, 0:1])

                # ---- head layer 2: this example's 4 L-chunks land in
                # one psum bank at partition rows 0/32/64/96 ------------
                psQ = ps_hq.tile([128, 512], F32, tag="phq")
                for c in range(4):
                    cs = slice(c * 512, (c + 1) * 512)
                    nc.tensor.matmul(psQ[32 * c:32 * c + 1, :],
                                     wtsb[:, 128:129],
                                     hid[:, cs], start=True, stop=True,
                                     tile_position=(0, 32 * c))
                # copy the contiguous 0..96 partition range (rows between
                # the 4 preds rows are dead) into the pair staging; alternate
                # ACT/DVE across the pair for engine balance
                if j == 0:
                    nc.scalar.activation(outS[0:97, 0:512], psQ[0:97, :],
                                         AF.Copy)
                else:
                    nc.vector.tensor_scalar(outS[0:97, 512:1024],
                                            psQ[0:97, :], 0.0, None, OP.add)
            # one out DMA per pair: dram (row j, chunk c) <- outS partition
            # 32c, free half j
            nc.sync.dma_start(
                out_d.ap()[e0:e0 + 2, :].rearrange("j (c x) -> c j x", c=4),
                outS[0:128:32, :].rearrange("p (j x) -> p j x", j=2))

    nc.compile()
    return nc


def kernel(x, t, mask_token,
           te_w1, te_b1, te_w2, te_b2,
           fproj_w, fproj_b, bproj_w, bproj_b,
           fwz, fbz, fwh, fbh,
           bwz, bbz, bwh, bbh,
           gh_w1, gh_b1, gh_w2, gh_b2):
    inp = dict(te_w1=te_w1, te_b1=te_b1, te_w2=te_w2, te_b2=te_b2,
               fproj_w=fproj_w, fproj_b=fproj_b, bproj_w=bproj_w,
               bproj_b=bproj_b, fwz=fwz, fbz=fbz, fwh=fwh, fbh=fbh,
               bwz=bwz, bbz=bbz, bwh=bwh, bbh=bbh,
               gh_w1=gh_w1, gh_b1=gh_b1, gh_w2=gh_w2, gh_b2=gh_b2)
    wts, wtsb, b2 = _pack_weights(inp)

    x = np.asarray(x, np.float32)
    t = np.asarray(t, np.float32)
    tok = np.asarray(mask_token, np.float32)
    xT = np.swapaxes(x, 1, 2)                    # (B, 3, L)
    mask = xT[:, 2:3, :]
    x12 = np.where(mask == 0, tok.reshape(1, 2, 1), xT[:, 0:2, :])
    tn = np.swapaxes(t, 1, 2)                    # (B, 1, L)
    inx = np.ascontiguousarray(
        np.concatenate([mask, x12, tn], axis=1), np.float32)  # (B, 4, L)
    mbf = np.ascontiguousarray(mask[:, 0, :]).astype(mybir.dt.np(BF16))

    if "nc" not in _cache:
        _cache["nc"] = _build_program()
    nc = _cache["nc"]

    in_maps = [
        {"wts": wts, "wtsb": wtsb, "inx": inx[c * BS:(c + 1) * BS],
         "mb": mbf[c * BS:(c + 1) * BS]}
        for c in range(NCORES)
    ]
    res = run_bass_kernel_spmd(nc, in_maps, core_ids=list(range(NCORES)))
    out = np.concatenate([res.results[c]["out"] for c in range(NCORES)], axis=0)
    return (out + b2).reshape(B, L, 1).astype(np.float32)
